# revision 7
# baseline (speedup 1.0000x reference)
"""Trainium2 Bass kernel: 16-head self-attention (B=4, S=2048, E=1024).

Reference math:
  Q = x @ W_q.T ; K = x @ W_k.T ; V = x @ W_v.T      (split into 16 heads of 64)
  A = softmax(Q K^T / sqrt(64)) ; Hout = A @ V
  out = concat_heads(Hout) @ W_o.T + b_o

Sharding: data-parallel over (batch, seq-half) -> 8 cores, no collectives.
Core i handles batch i//2 and query rows [ (i%2)*1024, (i%2+1)*1024 ).
K/V are computed for the full 2048-token sequence on every core (the two
cores sharing a batch duplicate the K/V projection work).  To keep one SPMD
program, odd cores receive x^T with the two sequence halves swapped so the
"first 1024 columns" are always the core's queries; softmax is permutation-
invariant over keys so K/V ordering doesn't matter.

On-chip layout: everything is kept feature-on-partition ("transposed"):
  xT  [e, t]  (host pre-transposed, bf16)
  WqT/WkT/WvT [e, o], WoT [o, u] (host pre-transposed, bf16)
  Q^T [o, t_q], K^T [o, t_k] via matmul(lhsT=W^T, rhs=xT)
  V stored head-interleaved with a ones column: VA[k, kt, h, 0:64]=V, [...,64]=1
  S^T[k, q] = matmul(lhsT=K^T_head, rhs=Q^T_head)   (contraction d=64)
  P = exp(S^T / 8)  (no max-subtraction needed: scores are N(0, ~0.33^2))
  O^T[d,q] + denom row = matmul(lhsT=VA_slice[128,65], rhs=P)
  Hout^T = O^T * (1/denom)  (denominator broadcast via K=1 matmul)
  Y[t, u] = matmul(lhsT=Hout^T tile, rhs=WoT) + b_o
"""

import sys

for _p in ("/opt/trn_rl_repo",):
    if _p not in sys.path:
        sys.path.append(_p)

import numpy as np
import ml_dtypes

import concourse.bass as bass
import concourse.mybir as mybir
import concourse.tile as tile
from concourse import bacc
from concourse.bass_utils import run_bass_kernel_spmd

B, S, E = 4, 2048, 1024
H, D = 16, 64
P = 128
SQ = S // 2  # queries per core
NCORES = 8
EC = E // P  # 8 feature chunks
KT_TILES = S // P  # 16 key tiles
QB = 512  # q block (matmul free dim / PSUM bank width)
KG = 2  # k-tiles per exp group (ACT instruction spans KG*512 psum cols)

BF16 = mybir.dt.bfloat16
F32 = mybir.dt.float32
EXP = mybir.ActivationFunctionType.Exp

_CACHE = {}


def _build():
    nc = bacc.Bacc("TRN2", target_bir_lowering=False, debug=False, num_devices=NCORES)

    xT = nc.dram_tensor("xT", [E, S], BF16, kind="ExternalInput").ap()
    wqT = nc.dram_tensor("wqT", [E, E], BF16, kind="ExternalInput").ap()
    wkT = nc.dram_tensor("wkT", [E, E], BF16, kind="ExternalInput").ap()
    wvT = nc.dram_tensor("wvT", [E, E], BF16, kind="ExternalInput").ap()
    woT = nc.dram_tensor("woT", [E, E], BF16, kind="ExternalInput").ap()
    b_o = nc.dram_tensor("b_o", [1, E], F32, kind="ExternalInput").ap()
    out = nc.dram_tensor("out", [SQ, E], F32, kind="ExternalOutput").ap()

    with tile.TileContext(nc) as tc:
        with tc.tile_pool(name="persist", bufs=1) as persist:
            QT = persist.tile([P, EC, SQ], BF16)       # Q^T  (o on partitions)
            KT = persist.tile([P, EC, S], BF16)        # K^T
            VA = persist.tile([P, KT_TILES, H, D + 1], BF16)  # V + ones col
            HT = persist.tile([P, EC, SQ], BF16)       # Hout^T
            wo_s = persist.tile([P, EC, E], BF16)
            bias_bc = persist.tile([P, E], F32)
            ones_bf = persist.tile([1, D], BF16)

            nc.vector.memset(ones_bf[:], 1.0)
            nc.vector.memset(VA[:, :, :, D:D + 1], 1.0)
            nc.sync.dma_start(wo_s[:], woT.rearrange("(c p) u -> p c u", p=P))

            # ---------------- phase 1: QKV projections ----------------
            with (
                tc.tile_pool(name="ld", bufs=1) as ld,
                tc.tile_pool(name="ps1", bufs=4, space="PSUM") as ps1,
            ):
                xTs = ld.tile([P, EC, S], BF16)
                nc.sync.dma_start(xTs[:], xT.rearrange("(c p) t -> p c t", p=P))
                wq_s = ld.tile([P, EC, E], BF16)
                nc.sync.dma_start(wq_s[:], wqT.rearrange("(c p) o -> p c o", p=P))
                wk_s = ld.tile([P, EC, E], BF16)
                nc.sync.dma_start(wk_s[:], wkT.rearrange("(c p) o -> p c o", p=P))
                wv_s = ld.tile([P, EC, E], BF16)
                nc.sync.dma_start(wv_s[:], wvT.rearrange("(c p) o -> p c o", p=P))

                # bias broadcast: [1,E] -> [128,E] via K=1 f32 matmuls (one-time)
                ones_f32 = ld.tile([1, P], F32)
                nc.vector.memset(ones_f32[:], 1.0)
                bo_s = ld.tile([1, E], F32)
                nc.sync.dma_start(bo_s[:], b_o)
                for ub in range(E // QB):
                    psb = ps1.tile([P, QB], F32, tag="ps")
                    nc.tensor.matmul(
                        psb[:], ones_f32[:], bo_s[:, ub * QB:(ub + 1) * QB],
                        start=True, stop=True,
                    )
                    nc.vector.tensor_copy(bias_bc[:, ub * QB:(ub + 1) * QB], psb[:])

                # Q^T (core's query half only) and K^T (full sequence)
                for c in range(EC):
                    for qb in range(SQ // QB):
                        ps = ps1.tile([P, QB], F32, tag="ps")
                        for ec in range(EC):
                            nc.tensor.matmul(
                                ps[:],
                                wq_s[:, ec, c * P:(c + 1) * P],
                                xTs[:, ec, qb * QB:(qb + 1) * QB],
                                start=(ec == 0), stop=(ec == EC - 1),
                            )
                        nc.vector.tensor_copy(QT[:, c, qb * QB:(qb + 1) * QB], ps[:])
                    for kb in range(S // QB):
                        ps = ps1.tile([P, QB], F32, tag="ps")
                        for ec in range(EC):
                            nc.tensor.matmul(
                                ps[:],
                                wk_s[:, ec, c * P:(c + 1) * P],
                                xTs[:, ec, kb * QB:(kb + 1) * QB],
                                start=(ec == 0), stop=(ec == EC - 1),
                            )
                        nc.vector.tensor_copy(KT[:, c, kb * QB:(kb + 1) * QB], ps[:])

                # V in natural [t, o] layout, scattered into VA head slices
                for tt in range(KT_TILES):
                    for ob in range(E // QB):
                        ps = ps1.tile([P, QB], F32, tag="ps")
                        for ec in range(EC):
                            nc.tensor.matmul(
                                ps[:],
                                xTs[:, ec, tt * P:(tt + 1) * P],
                                wv_s[:, ec, ob * QB:(ob + 1) * QB],
                                start=(ec == 0), stop=(ec == EC - 1),
                            )
                        for hh in range(QB // D):
                            h = ob * (QB // D) + hh
                            nc.vector.tensor_copy(
                                VA[:, tt, h, 0:D], ps[:, hh * D:(hh + 1) * D]
                            )

            # ---------------- phase 2: attention ----------------
            with (
                tc.tile_pool(name="attn_sb", bufs=3) as apool,
                tc.tile_pool(name="psS", bufs=3, space="PSUM") as psS_pool,
                tc.tile_pool(name="psO", bufs=2, space="PSUM") as psO_pool,
            ):
                for qb in range(SQ // QB):
                    q0 = qb * QB
                    for hp in range(EC):  # head pair (2 heads per 128-chunk)
                        psO = [
                            psO_pool.tile([D + 1, QB], F32, tag="psO", name=f"psO{hi}")
                            for hi in range(2)
                        ]
                        n_groups = KT_TILES // KG
                        for kg in range(n_groups):
                            psS = [
                                psS_pool.tile([P, KG, QB], F32, tag="psS", name=f"psS{hi}")
                                for hi in range(2)
                            ]
                            # scores: interleave the two heads so their
                            # K=64 matmuls pack rows 0-63 / 64-127 of the PE
                            for kt2 in range(KG):
                                kt = kg * KG + kt2
                                for hi in range(2):
                                    r0 = hi * D
                                    nc.tensor.matmul(
                                        psS[hi][:, kt2, :],
                                        KT[r0:r0 + D, hp, kt * P:(kt + 1) * P],
                                        QT[r0:r0 + D, hp, q0:q0 + QB],
                                        start=True, stop=True,
                                    )
                            probs = [None, None]
                            for hi in range(2):
                                probs[hi] = apool.tile(
                                    [P, KG, QB], BF16, tag="probs", name=f"probs{hi}"
                                )
                                nc.scalar.activation(
                                    probs[hi][:], psS[hi][:], EXP, scale=0.125
                                )
                            for hi in range(2):
                                h = hp * 2 + hi
                                for kt2 in range(KG):
                                    kt = kg * KG + kt2
                                    nc.tensor.matmul(
                                        psO[hi][:],
                                        VA[:, kt, h, :],
                                        probs[hi][:, kt2, :],
                                        start=(kg == 0 and kt2 == 0),
                                        stop=(kg == n_groups - 1 and kt2 == KG - 1),
                                    )
                        # normalize: Hout^T = O^T * (1/denom), denom = psO row D
                        for hi in range(2):
                            recip = apool.tile([1, QB], F32, tag="recip")
                            nc.vector.reciprocal(recip[:], psO[hi][D:D + 1, :])
                            recip_bf = apool.tile([1, QB], BF16, tag="recipbf")
                            nc.vector.tensor_copy(recip_bf[:], recip[:])
                            psb = psS_pool.tile([D, QB], F32, tag="psS")
                            nc.tensor.matmul(
                                psb[:], ones_bf[:], recip_bf[:],
                                start=True, stop=True,
                            )
                            rb_sb = apool.tile([D, QB], BF16, tag="rbsb")
                            nc.vector.tensor_copy(rb_sb[:], psb[:])
                            nc.vector.tensor_mul(
                                HT[hi * D:(hi + 1) * D, hp, q0:q0 + QB],
                                psO[hi][0:D, :],
                                rb_sb[:],
                            )

            # ---------------- phase 3: output projection ----------------
            with (
                tc.tile_pool(name="ysb", bufs=4) as ypool,
                tc.tile_pool(name="ps3", bufs=4, space="PSUM") as ps3,
            ):
                for tt in range(SQ // P):
                    for ub in range(E // QB):
                        ps = ps3.tile([P, QB], F32, tag="psy")
                        for oc in range(EC):
                            nc.tensor.matmul(
                                ps[:],
                                HT[:, oc, tt * P:(tt + 1) * P],
                                wo_s[:, oc, ub * QB:(ub + 1) * QB],
                                start=(oc == 0), stop=(oc == EC - 1),
                            )
                        y = ypool.tile([P, QB], F32, tag="y")
                        nc.vector.tensor_add(
                            y[:], ps[:], bias_bc[:, ub * QB:(ub + 1) * QB]
                        )
                        nc.sync.dma_start(
                            out[tt * P:(tt + 1) * P, ub * QB:(ub + 1) * QB], y[:]
                        )

    nc.compile()
    return nc


def get_nc():
    if "nc" not in _CACHE:
        _CACHE["nc"] = _build()
    return _CACHE["nc"]


def make_in_maps(x, W_q, W_k, W_v, W_o, b_o):
    bf16 = ml_dtypes.bfloat16
    wqT = np.ascontiguousarray(W_q.T).astype(bf16)
    wkT = np.ascontiguousarray(W_k.T).astype(bf16)
    wvT = np.ascontiguousarray(W_v.T).astype(bf16)
    woT = np.ascontiguousarray(W_o.T).astype(bf16)
    bo2 = np.ascontiguousarray(b_o.reshape(1, E)).astype(np.float32)

    in_maps = []
    for core in range(NCORES):
        b, half = core // 2, core % 2
        xb_T = np.ascontiguousarray(x[b].T)  # [E, S]
        if half == 1:
            # rotate so this core's queries are always columns [0, SQ)
            xb_T = np.concatenate([xb_T[:, SQ:], xb_T[:, :SQ]], axis=1)
        in_maps.append({
            "xT": np.ascontiguousarray(xb_T).astype(bf16),
            "wqT": wqT, "wkT": wkT, "wvT": wvT, "woT": woT,
            "b_o": bo2,
        })
    return in_maps


def run(x, W_q, W_k, W_v, W_o, b_o, **spmd_kwargs):
    nc = get_nc()
    in_maps = make_in_maps(x, W_q, W_k, W_v, W_o, b_o)
    res = run_bass_kernel_spmd(nc, in_maps, core_ids=list(range(NCORES)), **spmd_kwargs)
    out = np.empty((B, S, E), dtype=np.float32)
    for core in range(NCORES):
        b, half = core // 2, core % 2
        out[b, half * SQ:(half + 1) * SQ, :] = res.results[core]["out"]
    return out, res


def kernel(x, W_q, W_k, W_v, W_o, b_o):
    out, _ = run(x, W_q, W_k, W_v, W_o, b_o)
    return out


# revision 21
# speedup vs baseline: 1.4265x; 1.4265x over previous
"""Trainium2 Bass kernel: 16-head self-attention (B=4, S=2048, E=1024).

Reference math:
  Q = x @ W_q.T ; K = x @ W_k.T ; V = x @ W_v.T      (split into 16 heads of 64)
  A = softmax(Q K^T / sqrt(64)) ; Hout = A @ V
  out = concat_heads(Hout) @ W_o.T + b_o

Sharding: data-parallel over (batch, seq-half) -> 8 cores, no collectives.
Core i handles batch i//2 and query rows [ (i%2)*1024, (i%2+1)*1024 ).
K/V are computed for the full 2048-token sequence on every core (the two
cores sharing a batch duplicate the K/V projection work).  To keep one SPMD
program, odd cores receive x^T with the two sequence halves swapped so the
"first 1024 columns" are always the core's queries; softmax is permutation-
invariant over keys so K/V ordering doesn't matter.

On-chip layout: everything is kept feature-on-partition ("transposed"):
  xT  [e, t]  (host pre-transposed, bf16)
  WqT/WkT/WvT [e, o], WoT [o, u] (host pre-transposed, bf16)
  Q^T [o, t_q], K^T [o, t_k] via matmul(lhsT=W^T, rhs=xT)
  V stored head-interleaved with a ones column: VA[k, kt, h, 0:64]=V, [...,64]=1
  S^T[k, q] = matmul(lhsT=K^T_head, rhs=Q^T_head)   (contraction d=64)
  P = exp(S^T / 8)  (no max-subtraction needed: scores are N(0, ~0.33^2))
  O^T[d,q] + denom row = matmul(lhsT=VA_slice[128,65], rhs=P)
  Hout^T = O^T * (1/denom)  (denominator broadcast via K=1 matmul)
  Y[t, u] = matmul(lhsT=Hout^T tile, rhs=WoT) + b_o

Scheduling: the exp softmax runs on the Scalar/ACT engine and is nearly as
expensive as the attention matmuls; if the PE idles periodically the HAM
clock-gate drops it to 1.2 GHz (measured: the whole attention phase ran at
427ns/MM instead of 216).  So projection/output matmuls are interleaved into
the attention loop as filler to keep the PE continuously busy:
  phase A: K^T, V(heads 0-7), Q^T(qb0)         -- dense matmuls
  phase B: attention(qb0) + V(heads 8-15) + Q^T(qb1) filler
  phase C: attention(qb1) + output-projection(qb0 rows) filler
  phase D: output-projection(qb1 rows)
"""

import sys

for _p in ("/opt/trn_rl_repo",):
    if _p not in sys.path:
        sys.path.append(_p)

import numpy as np
import ml_dtypes

import concourse.bass as bass
import concourse.mybir as mybir
import concourse.tile as tile
from concourse import bacc
from concourse.bass_utils import run_bass_kernel_spmd

B, S, E = 4, 2048, 1024
H, D = 16, 64
P = 128
SQ = S // 2  # queries per core
NCORES = 8
EC = E // P  # 8 feature chunks
KT_TILES = S // P  # 16 key tiles
QB = 512  # q block (matmul free dim / PSUM bank width)
KG = 2  # k-tiles per exp group (ACT instruction spans KG*512 psum cols)
NQB = SQ // QB  # 2 q-blocks per core

BF16 = mybir.dt.bfloat16
F32 = mybir.dt.float32
EXP = mybir.ActivationFunctionType.Exp

_CACHE = {}


def _dma_chunked(nc, dst, src_2d):
    """DMA a [E, N] DRAM tensor into SBUF [P, EC, N], one chunk at a time so
    consumers of chunk 0 don't wait for the whole transfer."""
    r = src_2d.rearrange("(c p) t -> p c t", p=P)
    for c in range(EC):
        nc.sync.dma_start(dst[:, c], r[:, c])


def _build():
    nc = bacc.Bacc("TRN2", target_bir_lowering=False, debug=False, num_devices=NCORES)

    xT = nc.dram_tensor("xT", [E, S], BF16, kind="ExternalInput").ap()
    wqT = nc.dram_tensor("wqT", [E, E], BF16, kind="ExternalInput").ap()
    wkT = nc.dram_tensor("wkT", [E, E], BF16, kind="ExternalInput").ap()
    wvT = nc.dram_tensor("wvT", [E, E], BF16, kind="ExternalInput").ap()
    woT = nc.dram_tensor("woT", [E, E], BF16, kind="ExternalInput").ap()
    b_o = nc.dram_tensor("b_o", [1, E], F32, kind="ExternalInput").ap()
    out = nc.dram_tensor("out", [SQ, E], F32, kind="ExternalOutput").ap()

    with tile.TileContext(nc) as tc:
        with (
            tc.tile_pool(name="persist", bufs=1) as persist,
            tc.tile_pool(name="ld", bufs=1) as ld,
            tc.tile_pool(name="probs_sb", bufs=3) as ppool,
            tc.tile_pool(name="norm_sb", bufs=2) as apool,
            tc.tile_pool(name="ysb", bufs=2) as ypool,
            tc.tile_pool(name="ps1", bufs=2, space="PSUM") as ps1,
            tc.tile_pool(name="psS", bufs=2, space="PSUM") as psS_pool,
            tc.tile_pool(name="psO", bufs=2, space="PSUM") as psO_pool,
        ):
            QT = persist.tile([P, EC, SQ], BF16)       # Q^T  (o on partitions)
            KT = persist.tile([P, EC, S], BF16)        # K^T
            VA = persist.tile([P, KT_TILES, H, D + 1], BF16)  # V + ones col
            HT = persist.tile([P, EC, SQ], BF16)       # Hout^T
            bias_bc = persist.tile([P, E], F32)
            ones_bf = persist.tile([1, D], BF16)

            nc.vector.memset(ones_bf[:], 1.0)
            nc.vector.memset(VA[:, :, :, D:D + 1], 1.0)

            xTs = ld.tile([P, EC, S], BF16)
            _dma_chunked(nc, xTs, xT)
            wq_s = ld.tile([P, EC, E], BF16)
            _dma_chunked(nc, wq_s, wqT)

            def qproj_group(c, qb):
                """Q^T for output chunk c, q-block qb (8 MMs + 1 cast)."""
                ps = ps1.tile([P, QB], F32, tag="ps", name="psq")
                for ec in range(EC):
                    nc.tensor.matmul(
                        ps[:],
                        wq_s[:, ec, c * P:(c + 1) * P],
                        xTs[:, ec, qb * QB:(qb + 1) * QB],
                        start=(ec == 0), stop=(ec == EC - 1),
                    )
                nc.vector.tensor_copy(QT[:, c, qb * QB:(qb + 1) * QB], ps[:])

            def kproj_group(wk_s, c, kb):
                ps = ps1.tile([P, QB], F32, tag="ps", name="psk")
                for ec in range(EC):
                    nc.tensor.matmul(
                        ps[:],
                        wk_s[:, ec, c * P:(c + 1) * P],
                        xTs[:, ec, kb * QB:(kb + 1) * QB],
                        start=(ec == 0), stop=(ec == EC - 1),
                    )
                nc.vector.tensor_copy(KT[:, c, kb * QB:(kb + 1) * QB], ps[:])

            def vproj_group(wv_s, tt, ob):
                ps = ps1.tile([P, QB], F32, tag="ps", name="psv")
                for ec in range(EC):
                    nc.tensor.matmul(
                        ps[:],
                        xTs[:, ec, tt * P:(tt + 1) * P],
                        wv_s[:, ec, ob * QB:(ob + 1) * QB],
                        start=(ec == 0), stop=(ec == EC - 1),
                    )
                for hh in range(QB // D):
                    h = ob * (QB // D) + hh
                    nc.vector.tensor_copy(
                        VA[:, tt, h, 0:D], ps[:, hh * D:(hh + 1) * D]
                    )

            def outproj_group(wo_s, tt, ub):
                ps = ps1.tile([P, QB], F32, tag="ps", name="psy")
                for oc in range(EC):
                    nc.tensor.matmul(
                        ps[:],
                        HT[:, oc, tt * P:(tt + 1) * P],
                        wo_s[:, oc, ub * QB:(ub + 1) * QB],
                        start=(oc == 0), stop=(oc == EC - 1),
                    )
                y = ypool.tile([P, QB], F32, tag="y", name="y")
                nc.vector.tensor_add(y[:], ps[:], bias_bc[:, ub * QB:(ub + 1) * QB])
                nc.sync.dma_start(
                    out[tt * P:(tt + 1) * P, ub * QB:(ub + 1) * QB], y[:]
                )

            def attn_headpair(hp, qb):
                """Attention for head pair hp (heads 2hp, 2hp+1), q-block qb."""
                q0 = qb * QB
                psO = [
                    psO_pool.tile([D + 1, QB], F32, tag="psO", name=f"psO{hi}")
                    for hi in range(2)
                ]
                n_groups = KT_TILES // KG
                for kg in range(n_groups):
                    psS = [
                        psS_pool.tile([P, KG, QB], F32, tag="psS", name=f"psS{hi}")
                        for hi in range(2)
                    ]
                    for hi in range(2):
                        r0 = hi * D
                        for kt2 in range(KG):
                            kt = kg * KG + kt2
                            nc.tensor.matmul(
                                psS[hi][:, kt2, :],
                                KT[r0:r0 + D, hp, kt * P:(kt + 1) * P],
                                QT[r0:r0 + D, hp, q0:q0 + QB],
                                start=True, stop=True,
                            )
                    probs = [None, None]
                    for hi in range(2):
                        probs[hi] = ppool.tile(
                            [P, KG, QB], BF16, tag="probs", name=f"probs{hi}"
                        )
                        nc.scalar.activation(
                            probs[hi][:], psS[hi][:], EXP, scale=0.125
                        )
                    for hi in range(2):
                        h = hp * 2 + hi
                        for kt2 in range(KG):
                            kt = kg * KG + kt2
                            nc.tensor.matmul(
                                psO[hi][:],
                                VA[:, kt, h, :],
                                probs[hi][:, kt2, :],
                                start=(kg == 0 and kt2 == 0),
                                stop=(kg == n_groups - 1 and kt2 == KG - 1),
                            )
                # normalize: Hout^T = O^T * (1/denom), denom = psO row D
                for hi in range(2):
                    # custom-DVE ops require base partition 0: copy denom row out
                    dn = apool.tile([1, QB], F32, tag="dn", name="dn")
                    nc.vector.tensor_copy(dn[:], psO[hi][D:D + 1, :])
                    recip = apool.tile([1, QB], F32, tag="recip", name="recip")
                    nc.vector.reciprocal_approx_fast(recip[:], dn[:])
                    recip_bf = apool.tile([1, QB], BF16, tag="recipbf", name="recipbf")
                    nc.vector.tensor_copy(recip_bf[:], recip[:])
                    psb = ps1.tile([D, QB], F32, tag="ps", name="psrb")
                    nc.tensor.matmul(
                        psb[:], ones_bf[:], recip_bf[:], start=True, stop=True,
                    )
                    rb_sb = apool.tile([D, QB], BF16, tag="rbsb", name="rbsb")
                    nc.vector.tensor_copy(rb_sb[:], psb[:])
                    nc.vector.tensor_mul(
                        HT[hi * D:(hi + 1) * D, hp, q0:q0 + QB],
                        psO[hi][0:D, :],
                        rb_sb[:],
                    )

            with tc.tile_pool(name="ld_kv", bufs=1) as ld_kv:
                wk_s = ld_kv.tile([P, EC, E], BF16)
                _dma_chunked(nc, wk_s, wkT)
                wv_s = ld_kv.tile([P, EC, E], BF16)
                _dma_chunked(nc, wv_s, wvT)

                # ---------------- phase A: dense projections ----------------
                for c in range(EC):
                    qproj_group(c, 0)
                for c in range(EC):
                    for kb in range(S // QB):
                        kproj_group(wk_s, c, kb)
                for tt in range(KT_TILES):
                    vproj_group(wv_s, tt, 0)

                # ------------- phase B: attention(qb0) + filler -------------
                for hp in range(EC):
                    attn_headpair(hp, 0)
                    if hp < 4:  # V heads 8-15, done before head 8 (hp=4) needs
                        for tt in range(4 * hp, 4 * hp + 4):
                            vproj_group(wv_s, tt, 1)
                    if hp == 7:  # Q^T(qb1) chunk 0, needed first in phase C
                        qproj_group(0, 1)

            # wo / bias scope reuses the space freed by wk/wv
            with tc.tile_pool(name="ld_c", bufs=1) as ld_c:
                wo_s = ld_c.tile([P, EC, E], BF16)
                _dma_chunked(nc, wo_s, woT)

                # bias broadcast: [1,E] -> [128,E] via K=1 f32 matmuls
                ones_f32 = ld_c.tile([1, P], F32)
                nc.vector.memset(ones_f32[:], 1.0)
                bo_s = ld_c.tile([1, E], F32)
                nc.sync.dma_start(bo_s[:], b_o)
                for ub in range(E // QB):
                    psb = ps1.tile([P, QB], F32, tag="ps", name="psb")
                    nc.tensor.matmul(
                        psb[:], ones_f32[:], bo_s[:, ub * QB:(ub + 1) * QB],
                        start=True, stop=True,
                    )
                    nc.vector.tensor_copy(bias_bc[:, ub * QB:(ub + 1) * QB], psb[:])

                # ------------- phase C: attention(qb1) + filler -------------
                # filler: remaining Q^T(qb1) chunks + outproj of qb0 rows
                for hp in range(EC):
                    attn_headpair(hp, 1)
                    if hp < EC - 1:
                        qproj_group(hp + 1, 1)
                    outproj_group(wo_s, hp // 2, hp % 2)

                # ------------- phase D: outproj(qb1 rows, tiles 4-7) --------
                for tt in range(4, 8):
                    for ub in range(E // QB):
                        outproj_group(wo_s, tt, ub)

    nc.compile()
    return nc


def get_nc():
    if "nc" not in _CACHE:
        _CACHE["nc"] = _build()
    return _CACHE["nc"]


def make_in_maps(x, W_q, W_k, W_v, W_o, b_o):
    bf16 = ml_dtypes.bfloat16
    wqT = np.ascontiguousarray(W_q.T).astype(bf16)
    wkT = np.ascontiguousarray(W_k.T).astype(bf16)
    wvT = np.ascontiguousarray(W_v.T).astype(bf16)
    woT = np.ascontiguousarray(W_o.T).astype(bf16)
    bo2 = np.ascontiguousarray(b_o.reshape(1, E)).astype(np.float32)

    in_maps = []
    for core in range(NCORES):
        b, half = core // 2, core % 2
        xb_T = np.ascontiguousarray(x[b].T)  # [E, S]
        if half == 1:
            # rotate so this core's queries are always columns [0, SQ)
            xb_T = np.concatenate([xb_T[:, SQ:], xb_T[:, :SQ]], axis=1)
        in_maps.append({
            "xT": np.ascontiguousarray(xb_T).astype(bf16),
            "wqT": wqT, "wkT": wkT, "wvT": wvT, "woT": woT,
            "b_o": bo2,
        })
    return in_maps


def run(x, W_q, W_k, W_v, W_o, b_o, **spmd_kwargs):
    nc = get_nc()
    in_maps = make_in_maps(x, W_q, W_k, W_v, W_o, b_o)
    res = run_bass_kernel_spmd(nc, in_maps, core_ids=list(range(NCORES)), **spmd_kwargs)
    out = np.empty((B, S, E), dtype=np.float32)
    for core in range(NCORES):
        b, half = core // 2, core % 2
        out[b, half * SQ:(half + 1) * SQ, :] = res.results[core]["out"]
    return out, res


def kernel(x, W_q, W_k, W_v, W_o, b_o):
    out, _ = run(x, W_q, W_k, W_v, W_o, b_o)
    return out


# revision 24
# speedup vs baseline: 1.5307x; 1.0730x over previous
"""Trainium2 Bass kernel: 16-head self-attention (B=4, S=2048, E=1024).

Reference math:
  Q = x @ W_q.T ; K = x @ W_k.T ; V = x @ W_v.T      (split into 16 heads of 64)
  A = softmax(Q K^T / sqrt(64)) ; Hout = A @ V
  out = concat_heads(Hout) @ W_o.T + b_o

Sharding: data-parallel over (batch, seq-half) -> 8 cores, no collectives.
Core i handles batch i//2 and query rows [ (i%2)*1024, (i%2+1)*1024 ).
K/V are computed for the full 2048-token sequence on every core (the two
cores sharing a batch duplicate the K/V projection work).  To keep one SPMD
program, odd cores receive x^T with the two sequence halves swapped so the
"first 1024 columns" are always the core's queries; softmax is permutation-
invariant over keys so K/V ordering doesn't matter.

On-chip layout: everything is kept feature-on-partition ("transposed"):
  xT  [e, t]  (host pre-transposed, bf16)
  WqT/WkT/WvT [e, o], WoT [o, u] (host pre-transposed, bf16)
  Q^T [o, t_q], K^T [o, t_k] via matmul(lhsT=W^T, rhs=xT)
  V stored head-interleaved with a ones column: VA[k, kt, h, 0:64]=V, [...,64]=1
  S^T[k, q] = matmul(lhsT=K^T_head, rhs=Q^T_head)   (contraction d=64)
  P = exp(S^T / 8)  (no max-subtraction needed: scores are N(0, ~0.33^2))
  O^T[d,q] + denom row = matmul(lhsT=VA_slice[128,65], rhs=P)
  Hout^T = O^T * (1/denom)  (denominator broadcast via K=1 matmul)
  Y[t, u] = matmul(lhsT=Hout^T tile, rhs=WoT) + b_o

Scheduling: the exp softmax runs on the Scalar/ACT engine and is nearly as
expensive as the attention matmuls; if the PE idles periodically the HAM
clock-gate drops it to 1.2 GHz (measured: the whole attention phase ran at
427ns/MM instead of 216).  So projection/output matmuls are interleaved into
the attention loop as filler to keep the PE continuously busy:
  phase A: K^T, V(heads 0-7), Q^T(qb0)         -- dense matmuls
  phase B: attention(qb0) + V(heads 8-15) + Q^T(qb1) filler
  phase C: attention(qb1) + output-projection(qb0 rows) filler
  phase D: output-projection(qb1 rows)
"""

import sys

for _p in ("/opt/trn_rl_repo",):
    if _p not in sys.path:
        sys.path.append(_p)

import numpy as np
import ml_dtypes

import concourse.bass as bass
import concourse.mybir as mybir
import concourse.tile as tile
from concourse import bacc
from concourse.bass_utils import run_bass_kernel_spmd

B, S, E = 4, 2048, 1024
H, D = 16, 64
P = 128
SQ = S // 2  # queries per core
NCORES = 8
EC = E // P  # 8 feature chunks
KT_TILES = S // P  # 16 key tiles
QB = 512  # q block (matmul free dim / PSUM bank width)
KG = 2  # k-tiles per exp group (ACT instruction spans KG*512 psum cols)
NQB = SQ // QB  # 2 q-blocks per core

BF16 = mybir.dt.bfloat16
F32 = mybir.dt.float32
EXP = mybir.ActivationFunctionType.Exp

_CACHE = {}


def _dma_chunked(nc, dst, src_2d):
    """DMA a [E, N] DRAM tensor into SBUF [P, EC, N], one chunk at a time so
    consumers of chunk 0 don't wait for the whole transfer."""
    r = src_2d.rearrange("(c p) t -> p c t", p=P)
    for c in range(EC):
        nc.sync.dma_start(dst[:, c], r[:, c])


def _build():
    nc = bacc.Bacc("TRN2", target_bir_lowering=False, debug=False, num_devices=NCORES)

    xT = nc.dram_tensor("xT", [E, S], BF16, kind="ExternalInput").ap()
    wqT = nc.dram_tensor("wqT", [E, E], BF16, kind="ExternalInput").ap()
    wkT = nc.dram_tensor("wkT", [E, E], BF16, kind="ExternalInput").ap()
    wvT = nc.dram_tensor("wvT", [E, E], BF16, kind="ExternalInput").ap()
    woT = nc.dram_tensor("woT", [E, E], BF16, kind="ExternalInput").ap()
    b_o = nc.dram_tensor("b_o", [1, E], F32, kind="ExternalInput").ap()
    out = nc.dram_tensor("out", [SQ, E], F32, kind="ExternalOutput").ap()

    with tile.TileContext(nc) as tc:
        with (
            tc.tile_pool(name="persist", bufs=1) as persist,
            tc.tile_pool(name="ld", bufs=1) as ld,
            tc.tile_pool(name="probs_sb", bufs=3) as ppool,
            tc.tile_pool(name="norm_sb", bufs=2) as apool,
            tc.tile_pool(name="ysb", bufs=2) as ypool,
            tc.tile_pool(name="ps1", bufs=2, space="PSUM") as ps1,
            tc.tile_pool(name="psS", bufs=2, space="PSUM") as psS_pool,
            tc.tile_pool(name="psO", bufs=2, space="PSUM") as psO_pool,
        ):
            QT = persist.tile([P, EC, SQ], BF16)       # Q^T  (o on partitions)
            KT = persist.tile([P, EC, S], BF16)        # K^T
            VA = persist.tile([P, KT_TILES, H, D + 1], BF16)  # V + ones col
            HT = persist.tile([P, EC, SQ], BF16)       # Hout^T
            bias_bc = persist.tile([P, E], F32)
            ones_bf = persist.tile([1, D], BF16)

            nc.vector.memset(ones_bf[:], 1.0)
            nc.vector.memset(VA[:, :, :, D:D + 1], 1.0)

            xTs = ld.tile([P, EC, S], BF16)
            _dma_chunked(nc, xTs, xT)
            wq_s = ld.tile([P, EC, E], BF16)
            _dma_chunked(nc, wq_s, wqT)

            def qproj_group(c, qb):
                """Q^T for output chunk c, q-block qb (8 MMs + 1 cast)."""
                ps = ps1.tile([P, QB], F32, tag="ps", name="psq")
                for ec in range(EC):
                    nc.tensor.matmul(
                        ps[:],
                        wq_s[:, ec, c * P:(c + 1) * P],
                        xTs[:, ec, qb * QB:(qb + 1) * QB],
                        start=(ec == 0), stop=(ec == EC - 1),
                    )
                nc.vector.tensor_copy(QT[:, c, qb * QB:(qb + 1) * QB], ps[:])

            def kproj_group(wk_s, c, kb):
                ps = ps1.tile([P, QB], F32, tag="ps", name="psk")
                for ec in range(EC):
                    nc.tensor.matmul(
                        ps[:],
                        wk_s[:, ec, c * P:(c + 1) * P],
                        xTs[:, ec, kb * QB:(kb + 1) * QB],
                        start=(ec == 0), stop=(ec == EC - 1),
                    )
                nc.vector.tensor_copy(KT[:, c, kb * QB:(kb + 1) * QB], ps[:])

            def vproj_group(wv_s, tt, ob):
                ps = ps1.tile([P, QB], F32, tag="ps", name="psv")
                for ec in range(EC):
                    nc.tensor.matmul(
                        ps[:],
                        xTs[:, ec, tt * P:(tt + 1) * P],
                        wv_s[:, ec, ob * QB:(ob + 1) * QB],
                        start=(ec == 0), stop=(ec == EC - 1),
                    )
                for hh in range(QB // D):
                    h = ob * (QB // D) + hh
                    nc.vector.tensor_copy(
                        VA[:, tt, h, 0:D], ps[:, hh * D:(hh + 1) * D]
                    )

            def outproj_group(wo_s, tt, ub):
                ps = ps1.tile([P, QB], F32, tag="ps", name="psy")
                for oc in range(EC):
                    nc.tensor.matmul(
                        ps[:],
                        HT[:, oc, tt * P:(tt + 1) * P],
                        wo_s[:, oc, ub * QB:(ub + 1) * QB],
                        start=(oc == 0), stop=(oc == EC - 1),
                    )
                y = ypool.tile([P, QB], F32, tag="y", name="y")
                nc.vector.tensor_add(y[:], ps[:], bias_bc[:, ub * QB:(ub + 1) * QB])
                nc.sync.dma_start(
                    out[tt * P:(tt + 1) * P, ub * QB:(ub + 1) * QB], y[:]
                )

            def attn_headpair(hp, qb, fillers=None):
                """Attention for head pair hp (heads 2hp, 2hp+1), q-block qb.

                fillers: optional {kg: closure} of dense PE work emitted at the
                top of the given kg iteration — keeps the PE from idling (and
                the HAM clock-gate from re-throttling) while ACT runs exp.
                """
                q0 = qb * QB
                psO = [
                    psO_pool.tile([D + 1, QB], F32, tag="psO", name=f"psO{hi}")
                    for hi in range(2)
                ]
                n_groups = KT_TILES // KG
                for kg in range(n_groups):
                    if fillers and kg in fillers:
                        fillers[kg]()
                    psS = [
                        psS_pool.tile([P, KG, QB], F32, tag="psS", name=f"psS{hi}")
                        for hi in range(2)
                    ]
                    for hi in range(2):
                        r0 = hi * D
                        for kt2 in range(KG):
                            kt = kg * KG + kt2
                            nc.tensor.matmul(
                                psS[hi][:, kt2, :],
                                KT[r0:r0 + D, hp, kt * P:(kt + 1) * P],
                                QT[r0:r0 + D, hp, q0:q0 + QB],
                                start=True, stop=True,
                            )
                    probs = [None, None]
                    for hi in range(2):
                        probs[hi] = ppool.tile(
                            [P, KG, QB], BF16, tag="probs", name=f"probs{hi}"
                        )
                        nc.scalar.activation(
                            probs[hi][:], psS[hi][:], EXP, scale=0.125
                        )
                    for hi in range(2):
                        h = hp * 2 + hi
                        for kt2 in range(KG):
                            kt = kg * KG + kt2
                            nc.tensor.matmul(
                                psO[hi][:],
                                VA[:, kt, h, :],
                                probs[hi][:, kt2, :],
                                start=(kg == 0 and kt2 == 0),
                                stop=(kg == n_groups - 1 and kt2 == KG - 1),
                            )
                # normalize: Hout^T = O^T * (1/denom), denom = psO row D
                for hi in range(2):
                    # custom-DVE ops require base partition 0: copy denom row out
                    dn = apool.tile([1, QB], F32, tag="dn", name="dn")
                    nc.vector.tensor_copy(dn[:], psO[hi][D:D + 1, :])
                    recip = apool.tile([1, QB], F32, tag="recip", name="recip")
                    nc.vector.reciprocal_approx_fast(recip[:], dn[:])
                    recip_bf = apool.tile([1, QB], BF16, tag="recipbf", name="recipbf")
                    nc.vector.tensor_copy(recip_bf[:], recip[:])
                    psb = ps1.tile([D, QB], F32, tag="ps", name="psrb")
                    nc.tensor.matmul(
                        psb[:], ones_bf[:], recip_bf[:], start=True, stop=True,
                    )
                    rb_sb = apool.tile([D, QB], BF16, tag="rbsb", name="rbsb")
                    nc.vector.tensor_copy(rb_sb[:], psb[:])
                    nc.vector.tensor_mul(
                        HT[hi * D:(hi + 1) * D, hp, q0:q0 + QB],
                        psO[hi][0:D, :],
                        rb_sb[:],
                    )

            with tc.tile_pool(name="ld_kv", bufs=1) as ld_kv:
                wk_s = ld_kv.tile([P, EC, E], BF16)
                _dma_chunked(nc, wk_s, wkT)
                wv_s = ld_kv.tile([P, EC, E], BF16)
                _dma_chunked(nc, wv_s, wvT)

                # ---------------- phase A: dense projections ----------------
                # kproj kb=3 (all but hp4) and kb=2 (hp5-7) are deferred into
                # phase B as attention filler; attention(hp) only reads those
                # K columns at kg>=4, and the filler lands at kg<=3 of the
                # same hp.
                for c in range(EC):
                    qproj_group(c, 0)
                for c in range(EC):
                    for kb in range(S // QB):
                        if kb == 3 and c != 4:
                            continue
                        if kb == 2 and c >= 5:
                            continue
                        kproj_group(wk_s, c, kb)
                for tt in range(KT_TILES):
                    vproj_group(wv_s, tt, 0)

                # ------------- phase B: attention(qb0) + filler -------------
                # V heads 8-15 (16 groups) spread across hp0-4; vproj(tt,1)
                # must land before PV of head>=8 reads k-tile tt (hp4, kg
                # tt//KG) -- fillers run at the top of their kg.
                def fB(hp):
                    d = {}
                    if hp < 4:
                        d[1] = lambda: kproj_group(wk_s, hp, 3)
                        d[3] = lambda: vproj_group(wv_s, 3 * hp + 0, 1)
                        d[5] = lambda: vproj_group(wv_s, 3 * hp + 1, 1)
                        d[7] = lambda: vproj_group(wv_s, 3 * hp + 2, 1)
                    elif hp == 4:
                        d[0] = lambda: vproj_group(wv_s, 12, 1)
                        d[2] = lambda: vproj_group(wv_s, 13, 1)
                        d[4] = lambda: vproj_group(wv_s, 14, 1)
                        d[6] = lambda: vproj_group(wv_s, 15, 1)
                    else:
                        d[1] = lambda: kproj_group(wk_s, hp, 3)
                        d[3] = lambda: kproj_group(wk_s, hp, 2)
                        if hp == 5:
                            d[5] = lambda: qproj_group(0, 1)
                    return d

                for hp in range(EC):
                    attn_headpair(hp, 0, fB(hp))

            # wo / bias scope reuses the space freed by wk/wv
            with tc.tile_pool(name="ld_c", bufs=1) as ld_c:
                wo_s = ld_c.tile([P, EC, E], BF16)
                _dma_chunked(nc, wo_s, woT)

                # bias broadcast: [1,E] -> [128,E] via K=1 f32 matmuls
                ones_f32 = ld_c.tile([1, P], F32)
                nc.vector.memset(ones_f32[:], 1.0)
                bo_s = ld_c.tile([1, E], F32)
                nc.sync.dma_start(bo_s[:], b_o)
                for ub in range(E // QB):
                    psb = ps1.tile([P, QB], F32, tag="ps", name="psb")
                    nc.tensor.matmul(
                        psb[:], ones_f32[:], bo_s[:, ub * QB:(ub + 1) * QB],
                        start=True, stop=True,
                    )
                    nc.vector.tensor_copy(bias_bc[:, ub * QB:(ub + 1) * QB], psb[:])

                # ------------- phase C: attention(qb1) + filler -------------
                # filler: remaining Q^T(qb1) chunks + outproj of qb0 rows
                def fC(hp):
                    d = {}
                    if hp < EC - 1:
                        d[1] = lambda: qproj_group(hp + 1, 1)
                    d[4] = lambda: outproj_group(wo_s, hp // 2, hp % 2)
                    return d

                for hp in range(EC):
                    attn_headpair(hp, 1, fC(hp))

                # ------------- phase D: outproj(qb1 rows, tiles 4-7) --------
                for tt in range(4, 8):
                    for ub in range(E // QB):
                        outproj_group(wo_s, tt, ub)

    nc.compile()
    return nc


def get_nc():
    if "nc" not in _CACHE:
        _CACHE["nc"] = _build()
    return _CACHE["nc"]


def make_in_maps(x, W_q, W_k, W_v, W_o, b_o):
    bf16 = ml_dtypes.bfloat16
    wqT = np.ascontiguousarray(W_q.T).astype(bf16)
    wkT = np.ascontiguousarray(W_k.T).astype(bf16)
    wvT = np.ascontiguousarray(W_v.T).astype(bf16)
    woT = np.ascontiguousarray(W_o.T).astype(bf16)
    bo2 = np.ascontiguousarray(b_o.reshape(1, E)).astype(np.float32)

    in_maps = []
    for core in range(NCORES):
        b, half = core // 2, core % 2
        xb_T = np.ascontiguousarray(x[b].T)  # [E, S]
        if half == 1:
            # rotate so this core's queries are always columns [0, SQ)
            xb_T = np.concatenate([xb_T[:, SQ:], xb_T[:, :SQ]], axis=1)
        in_maps.append({
            "xT": np.ascontiguousarray(xb_T).astype(bf16),
            "wqT": wqT, "wkT": wkT, "wvT": wvT, "woT": woT,
            "b_o": bo2,
        })
    return in_maps


def run(x, W_q, W_k, W_v, W_o, b_o, **spmd_kwargs):
    nc = get_nc()
    in_maps = make_in_maps(x, W_q, W_k, W_v, W_o, b_o)
    res = run_bass_kernel_spmd(nc, in_maps, core_ids=list(range(NCORES)), **spmd_kwargs)
    out = np.empty((B, S, E), dtype=np.float32)
    for core in range(NCORES):
        b, half = core // 2, core % 2
        out[b, half * SQ:(half + 1) * SQ, :] = res.results[core]["out"]
    return out, res


def kernel(x, W_q, W_k, W_v, W_o, b_o):
    out, _ = run(x, W_q, W_k, W_v, W_o, b_o)
    return out
